# revision 1
# baseline (speedup 1.0000x reference)
"""EngagementPredictor TRN2 kernel: 3-branch MHA + masked mean-pool + MLP.

Sharding: pure data-parallel — B=8 batch elements, one per NeuronCore;
weights replicated; no collectives. Each core computes its [2]-logit row.

Per-core dataflow (S=1024, H=1024, fp32 storage / float32r matmuls):
  xT [H,S] resident in SBUF. For each MHA branch (beh 8h / tmp 4h / pat 4h):
    QT,KT [H,S] = W^T-stationary projections (Q gets its bias fused into the
    PSUM->SBUF evacuation; K bias dropped — softmax is invariant to per-q
    constants; V bias folded into the pooled vector).
    V [S,H] natural-layout projection.
    Attention in transposed layout: scoresT[k,q] per (head, q-chunk, k-tile),
    Exp fused with 1/sqrt(d) scale and the key mask as a per-partition bias
    (-30000 on masked keys -> exact zero probs). ctxT[d,q] = V^T @ expT needs
    no transposes. Softmax denominator via ones-column matmuls; the pooling
    weights mask[q]/(mask_sum*denom[q]) are broadcast across partitions with a
    K=1 matmul; masked mean-pool runs on DVE directly off PSUM.
    o-projection applied AFTER pooling (1xH instead of SxH).
    fus1 partial products accumulate per-branch so fus1_w streams during
    compute instead of serializing at the tail.
  Tail: relu MLP (fus1/fus2/cls) at M=1.
"""
import numpy as np

import concourse.bass as bass
import concourse.tile as tile
from concourse import mybir
from concourse.bass_utils import run_bass_kernel_spmd

F32 = mybir.dt.float32
F32R = mybir.dt.float32r
AF = mybir.ActivationFunctionType
ALU = mybir.AluOpType

P = 128
S = 1024
H = 1024
NT = H // P          # 8 tiles of 128 along H or S
QC = 512             # chunk width for projections / o-proj / MLP
NQC = S // QC        # 2
QCA = 256            # attention q-chunk width (SBUF-budget bound)
NQCA = S // QCA      # 4
NCORES = 8
MHAS = [("beh", 8), ("tmp", 4), ("pat", 4)]

_CACHE = {}


def _build_nc():
    nc = bass.Bass()
    dram = {}

    def dp(name, shape):
        dram[name] = nc.declare_dram_parameter(name, list(shape), F32,
                                               isOutput=False)

    dp("xT", (H, S))
    dp("maskb", (P, NT))       # -30000/0 per key position, partition-inner
    dp("poolw", (1, S))        # mask / mask_sum
    dp("ones", (P,))
    for m, _ in MHAS:
        for wn in ("qw", "kw", "vw", "ow"):
            dp(f"{m}_{wn}", (H, H))
        dp(f"{m}_qb", (P, NT))
        dp(f"{m}_vb", (P, NT))
        dp(f"{m}_ob", (P, NT))
    dp("fus1_w", (3 * H, H))
    dp("fus1_b", (P, NT))
    dp("fus2_w", (H, H // 2))
    dp("fus2_b", (P, 4))
    dp("cls_w", (H // 2, 2))
    dp("cls_b", (1, 2))
    out = nc.declare_dram_parameter("out", [1, 2], F32, isOutput=True)

    def r3(ap):  # [K, N] dram -> [P, K//P, N] partition-inner
        return ap[:].rearrange("(t p) n -> p t n", p=P)

    with tile.TileContext(nc) as tc, \
         nc.allow_low_precision(
             reason="float32r tiles: fp32 bits with mantissa rounding on "
                    "write; DVE reduces accumulate at fp32 internally"):
        with tc.tile_pool(name="big", bufs=1) as big, \
             tc.tile_pool(name="wstr", bufs=3) as wstr, \
             tc.tile_pool(name="expp", bufs=2) as expp, \
             tc.tile_pool(name="small", bufs=1) as small, \
             tc.tile_pool(name="work", bufs=2) as work:

            # ---- resident inputs ----
            xT = big.tile([P, NT, S], F32R, tag="xT")
            nc.sync.dma_start(xT[:], r3(dram["xT"]).bitcast(F32R))
            QT = big.tile([P, NT, S], F32R, tag="QT")
            KT = big.tile([P, NT, S], F32R, tag="KT")
            V = big.tile([P, NT, H], F32R, tag="V")

            mb = small.tile([P, NT], F32, tag="mb")
            nc.sync.dma_start(mb[:], dram["maskb"][:])
            pw = small.tile([1, S], F32, tag="pw")
            nc.sync.dma_start(pw[:], dram["poolw"][:])
            ones_col = small.tile([P, 1], F32R, tag="ones_col")
            nc.sync.dma_start(ones_col[:], dram["ones"][:, None].bitcast(F32R))
            ones_row = small.tile([1, P], F32R, tag="ones_row")
            nc.sync.dma_start(ones_row[:], dram["ones"][None, :].bitcast(F32R))

            # h1 pre-activation accumulated in column layout [P, NT]
            h1acc = small.tile([P, NT], F32, tag="h1acc")
            nc.vector.memset(h1acc[:], 0.0)

            for mi, (m, nh) in enumerate(MHAS):
                d = H // nh
                ndt = d // P
                inv_sqrt_d = 1.0 / float(np.sqrt(d))

                qb = small.tile([P, NT], F32, tag="qb")
                nc.sync.dma_start(qb[:], dram[f"{m}_qb"][:])
                vb = small.tile([P, NT], F32, tag="vb")
                nc.sync.dma_start(vb[:], dram[f"{m}_vb"][:])
                ob = small.tile([P, NT], F32, tag="ob")
                nc.sync.dma_start(ob[:], dram[f"{m}_ob"][:])

                # ---------- projections ----------
                with tc.tile_pool(name=f"pj{mi}", bufs=8, space="PSUM") as pj:
                    # Q and K: out[ho, s] ; lhsT = w[ki, ho-slice] (stationary)
                    for wn, dst, with_bias in ((f"{m}_qw", QT, True),
                                               (f"{m}_kw", KT, False)):
                        wr = r3(dram[wn]).bitcast(F32R)
                        for hog in range(2):
                            pst = [pj.tile([P, QC], F32, tag="pj",
                                           name=f"pj{mi}_{wn}_{hog}_{i}")
                                   for i in range(8)]
                            for ki in range(NT):
                                wt = wstr.tile([P, H], F32R, tag="w")
                                nc.sync.dma_start(wt[:], wr[:, ki])
                                for ho4 in range(4):
                                    ho = hog * 4 + ho4
                                    hsl = slice(ho * P, (ho + 1) * P)
                                    for qc in range(NQC):
                                        qsl = slice(qc * QC, (qc + 1) * QC)
                                        nc.tensor.matmul(
                                            pst[ho4 * 2 + qc][:],
                                            lhsT=wt[:, hsl],
                                            rhs=xT[:, ki, qsl],
                                            start=(ki == 0),
                                            stop=(ki == NT - 1))
                            for ho4 in range(4):
                                ho = hog * 4 + ho4
                                for qc in range(NQC):
                                    qsl = slice(qc * QC, (qc + 1) * QC)
                                    pt = pst[ho4 * 2 + qc]
                                    if with_bias:
                                        nc.scalar.activation(
                                            dst[:, ho, qsl], pt[:], AF.Identity,
                                            bias=qb[:, ho:ho + 1], scale=1.0)
                                    else:
                                        nc.vector.tensor_copy(
                                            dst[:, ho, qsl], pt[:])
                    # V: out[s, h] ; lhsT = xT[ki, s-slice] (stationary)
                    vr = r3(dram[f"{m}_vw"]).bitcast(F32R)
                    for sg in range(2):
                        pst = [pj.tile([P, QC], F32, tag="pj",
                                       name=f"pjv{mi}_{sg}_{i}")
                               for i in range(8)]
                        for ki in range(NT):
                            wt = wstr.tile([P, H], F32R, tag="w")
                            nc.sync.dma_start(wt[:], vr[:, ki])
                            for s4 in range(4):
                                st = sg * 4 + s4
                                ssl = slice(st * P, (st + 1) * P)
                                for hc in range(NQC):
                                    hsl = slice(hc * QC, (hc + 1) * QC)
                                    nc.tensor.matmul(
                                        pst[s4 * 2 + hc][:],
                                        lhsT=xT[:, ki, ssl],
                                        rhs=wt[:, hsl],
                                        start=(ki == 0),
                                        stop=(ki == NT - 1))
                        for s4 in range(4):
                            st = sg * 4 + s4
                            for hc in range(NQC):
                                hsl = slice(hc * QC, (hc + 1) * QC)
                                nc.vector.tensor_copy(
                                    V[:, st, hsl], pst[s4 * 2 + hc][:])

                # ---------- attention + pool + o-proj + fus1 partial ----------
                with tc.tile_pool(name=f"sc{mi}", bufs=2, space="PSUM") as psc, \
                     tc.tile_pool(name=f"cx{mi}", bufs=1, space="PSUM") as pcx, \
                     tc.tile_pool(name=f"dn{mi}", bufs=1, space="PSUM") as pdn, \
                     tc.tile_pool(name=f"wb{mi}", bufs=1, space="PSUM") as pwb, \
                     tc.tile_pool(name=f"po{mi}", bufs=2, space="PSUM") as ppo:
                    pooled = small.tile([P, NT], F32R, tag="pooled")
                    for qc in range(NQCA):
                        qsl = slice(qc * QCA, (qc + 1) * QCA)
                        for h in range(nh):
                            expt = expp.tile([P, NT, QCA], F32R, tag="expt")
                            for kt in range(NT):
                                ksl = slice(kt * P, (kt + 1) * P)
                                ssc = psc.tile([P, QCA], F32, tag="sc")
                                for dt in range(ndt):
                                    nc.tensor.matmul(
                                        ssc[:],
                                        lhsT=KT[:, h * ndt + dt, ksl],
                                        rhs=QT[:, h * ndt + dt, qsl],
                                        start=(dt == 0),
                                        stop=(dt == ndt - 1))
                                nc.scalar.activation(
                                    expt[:, kt], ssc[:], AF.Exp,
                                    bias=mb[:, kt:kt + 1], scale=inv_sqrt_d)
                            sdn = pdn.tile([1, QCA], F32, tag="dn")
                            for kt in range(NT):
                                nc.tensor.matmul(
                                    sdn[:], lhsT=ones_col[:], rhs=expt[:, kt],
                                    start=(kt == 0), stop=(kt == NT - 1))
                            recip = work.tile([1, QCA], F32, tag="recip")
                            nc.vector.reciprocal(recip[:], sdn[:])
                            w = work.tile([1, QCA], F32R, tag="w")
                            nc.vector.tensor_mul(out=w[:], in0=recip[:],
                                                 in1=pw[:, qsl])
                            swb = pwb.tile([P, QCA], F32, tag="wb")
                            nc.tensor.matmul(swb[:], lhsT=ones_row[:],
                                             rhs=w[:], start=True, stop=True)
                            wb_sb = work.tile([P, QCA], F32, tag="wb_sb")
                            nc.vector.tensor_copy(wb_sb[:], swb[:])
                            for dt in range(ndt):
                                gdt = h * ndt + dt
                                dsl = slice(gdt * P, (gdt + 1) * P)
                                sctx = pcx.tile([P, QCA], F32, tag="cx")
                                for kt in range(NT):
                                    nc.tensor.matmul(
                                        sctx[:], lhsT=V[:, kt, dsl],
                                        rhs=expt[:, kt],
                                        start=(kt == 0), stop=(kt == NT - 1))
                                prod = work.tile([P, QCA], F32, tag="prod")
                                nc.vector.tensor_mul(out=prod[:], in0=sctx[:],
                                                     in1=wb_sb[:])
                                if qc == 0:
                                    nc.vector.tensor_reduce(
                                        pooled[:, gdt:gdt + 1], prod[:],
                                        axis=mybir.AxisListType.X, op=ALU.add)
                                else:
                                    pp = work.tile([P, 1], F32, tag="pp")
                                    nc.vector.tensor_reduce(
                                        pp[:], prod[:],
                                        axis=mybir.AxisListType.X, op=ALU.add)
                                    nc.vector.tensor_add(
                                        out=pooled[:, gdt:gdt + 1],
                                        in0=pooled[:, gdt:gdt + 1], in1=pp[:])
                    # + V bias (exact: pooling weights sum to 1)
                    nc.vector.tensor_add(out=pooled[:], in0=pooled[:],
                                         in1=vb[:])
                    # o-projection, column layout: fTm[p,t] = (pooled@ow)[t*P+p]
                    # lhsT = ow k-tile column block (stationary), rhs = pooled
                    # column (N=1). ob fused into the PSUM evacuation.
                    owr = r3(dram[f"{m}_ow"]).bitcast(F32R)
                    fTm = small.tile([P, NT], F32R, tag="fTm")
                    for tg in range(4):
                        pos = [ppo.tile([P, 1], F32, tag="po",
                                        name=f"po{mi}_{tg}_{i}")
                               for i in range(2)]
                        for ki in range(NT):
                            owt = wstr.tile([P, H], F32R, tag="w")
                            nc.sync.dma_start(owt[:], owr[:, ki])
                            for t2 in range(2):
                                t = tg * 2 + t2
                                nc.tensor.matmul(
                                    pos[t2][:],
                                    lhsT=owt[:, t * P:(t + 1) * P]
                                    .bitcast(F32),
                                    rhs=pooled[:, ki:ki + 1].bitcast(F32),
                                    start=(ki == 0), stop=(ki == NT - 1))
                        for t2 in range(2):
                            t = tg * 2 + t2
                            nc.scalar.activation(
                                fTm[:, t:t + 1], pos[t2][:], AF.Identity,
                                bias=ob[:, t:t + 1], scale=1.0)
                    # fus1 partial: h1acc += fused[m-part] @ fus1_w[m-rows]
                    w1r = r3(dram["fus1_w"]).bitcast(F32R)
                    for tg in range(4):
                        ph1 = [ppo.tile([P, 1], F32, tag="po",
                                        name=f"ph1_{mi}_{tg}_{i}")
                               for i in range(2)]
                        for ki in range(NT):
                            w1t = wstr.tile([P, H], F32R, tag="w")
                            nc.sync.dma_start(w1t[:], w1r[:, mi * NT + ki])
                            for t2 in range(2):
                                t = tg * 2 + t2
                                nc.tensor.matmul(
                                    ph1[t2][:],
                                    lhsT=w1t[:, t * P:(t + 1) * P]
                                    .bitcast(F32),
                                    rhs=fTm[:, ki:ki + 1].bitcast(F32),
                                    start=(ki == 0), stop=(ki == NT - 1))
                        for t2 in range(2):
                            t = tg * 2 + t2
                            nc.vector.tensor_add(
                                out=h1acc[:, t:t + 1], in0=ph1[t2][:],
                                in1=h1acc[:, t:t + 1])

            # ---------- MLP tail ----------
            with tc.tile_pool(name="tail", bufs=2, space="PSUM") as ptl:
                b1 = small.tile([P, NT], F32, tag="b1")
                nc.sync.dma_start(b1[:], dram["fus1_b"][:])
                h1pre = small.tile([P, NT], F32, tag="h1pre")
                nc.vector.tensor_add(out=h1pre[:], in0=h1acc[:], in1=b1[:])
                h1T = small.tile([P, NT], F32R, tag="h1T")
                nc.scalar.activation(h1T[:], h1pre[:], AF.Relu)

                w2r = r3(dram["fus2_w"]).bitcast(F32R)  # [P, 8, 512]
                b2 = small.tile([P, 4], F32, tag="b2")
                nc.sync.dma_start(b2[:], dram["fus2_b"][:])
                h2T = small.tile([P, 4], F32R, tag="h2T")
                for tg in range(2):
                    ph2 = [ptl.tile([P, 1], F32, tag="t2",
                                    name=f"ph2_{tg}_{i}") for i in range(2)]
                    for ki in range(NT):
                        w2t = wstr.tile([P, QC], F32R, tag="w2")
                        nc.sync.dma_start(w2t[:], w2r[:, ki])
                        for t2 in range(2):
                            t = tg * 2 + t2
                            nc.tensor.matmul(
                                ph2[t2][:],
                                lhsT=w2t[:, t * P:(t + 1) * P].bitcast(F32),
                                rhs=h1T[:, ki:ki + 1].bitcast(F32),
                                start=(ki == 0), stop=(ki == NT - 1))
                    for t2 in range(2):
                        t = tg * 2 + t2
                        nc.scalar.activation(h2T[:, t:t + 1], ph2[t2][:],
                                             AF.Relu, bias=b2[:, t:t + 1],
                                             scale=1.0)

                cwr = r3(dram["cls_w"]).bitcast(F32R)  # [P, 4, 2]
                cwt = small.tile([P, 4, 2], F32R, tag="cwt")
                nc.sync.dma_start(cwt[:], cwr)
                plg = ptl.tile([1, 2], F32, tag="lg")
                for ki in range(4):
                    nc.tensor.matmul(plg[:],
                                     lhsT=h2T[:, ki:ki + 1].bitcast(F32),
                                     rhs=cwt[:, ki].bitcast(F32),
                                     start=(ki == 0), stop=(ki == 3))
                cb = small.tile([1, 2], F32, tag="cb")
                nc.sync.dma_start(cb[:], dram["cls_b"][:])
                lg = small.tile([1, 2], F32, tag="lgsb")
                nc.vector.tensor_add(out=lg[:], in0=plg[:], in1=cb[:])
                nc.sync.dma_start(out[:], lg[:])

    _split_multi_waits(nc)
    return nc


def _split_multi_waits(nc, max_on_inst=1, max_on_evsem=2):
    """This walrus build caps sync waits per instruction at 1 (2 for
    EventSemaphore); Tile attaches one wait per dependent proc. Spill excess
    waits onto pure-wait EventSemaphores inserted before, on the same engine —
    the engine blocks on each condition in sequence, so semantics match."""
    for f in nc.m.functions:
        for bb in f.blocks:
            insts = list(bb.instructions)
            new = []
            changed = False
            for ins in insts:
                si = ins.sync_info
                if si is not None:
                    waits = list(si.on_wait)
                    cap = (max_on_evsem
                           if isinstance(ins, mybir.InstEventSemaphore)
                           else max_on_inst)
                    if len(waits) > cap:
                        spill = waits[:-cap]
                        keep = waits[-cap:]
                        k = 0
                        while spill:
                            chunk = spill[:max_on_evsem]
                            spill = spill[max_on_evsem:]
                            new.append(mybir.InstEventSemaphore(
                                name=f"{ins.name}-wspill{k}",
                                engine=ins.engine, ins=[], outs=[],
                                sync_info=mybir.SyncInfo(on_wait=chunk,
                                                         on_update=[])))
                            k += 1
                        ins.sync_info = mybir.SyncInfo(
                            on_wait=keep, on_update=list(si.on_update))
                        changed = True
                new.append(ins)
            if changed:
                bb.instructions = new


def _get_nc():
    if "nc" not in _CACHE:
        _CACHE["nc"] = _build_nc()
    return _CACHE["nc"]


def _prep_in_maps(inputs):
    f32 = np.float32
    mask = inputs["attention_mask"].astype(f32)          # [B, S]
    denom = mask.sum(axis=1, keepdims=True)              # [B, 1]
    poolw = (mask / denom).astype(f32)                   # [B, S]
    maskb = np.where(mask > 0, 0.0, -30000.0).astype(f32)  # [B, S]

    shared = {"ones": np.ones(P, f32)}
    for m, _ in MHAS:
        for wn in ("qw", "kw", "vw", "ow"):
            shared[f"{m}_{wn}"] = np.ascontiguousarray(
                inputs[f"{m}_{wn}"], dtype=f32)
        shared[f"{m}_qb"] = np.ascontiguousarray(
            inputs[f"{m}_qb"].astype(f32).reshape(NT, P).T)
        shared[f"{m}_vb"] = np.ascontiguousarray(
            inputs[f"{m}_vb"].astype(f32).reshape(NT, P).T)
        shared[f"{m}_ob"] = np.ascontiguousarray(
            inputs[f"{m}_ob"].astype(f32).reshape(NT, P).T)
    shared["fus1_w"] = np.ascontiguousarray(inputs["fus1_w"], dtype=f32)
    shared["fus1_b"] = np.ascontiguousarray(
        inputs["fus1_b"].astype(f32).reshape(NT, P).T)
    shared["fus2_w"] = np.ascontiguousarray(inputs["fus2_w"], dtype=f32)
    shared["fus2_b"] = np.ascontiguousarray(
        inputs["fus2_b"].astype(f32).reshape(4, P).T)
    shared["cls_w"] = np.ascontiguousarray(inputs["cls_w"], dtype=f32)
    shared["cls_b"] = inputs["cls_b"].astype(f32).reshape(1, 2)

    in_maps = []
    for c in range(NCORES):
        im = dict(shared)
        im["xT"] = np.ascontiguousarray(
            inputs["hidden_states"][c].astype(f32).T)
        im["maskb"] = np.ascontiguousarray(maskb[c].reshape(NT, P).T)
        im["poolw"] = poolw[c].reshape(1, S)
        in_maps.append(im)
    return in_maps


def kernel(**inputs) -> np.ndarray:
    nc = _get_nc()
    in_maps = _prep_in_maps(inputs)
    res = run_bass_kernel_spmd(nc, in_maps, core_ids=list(range(NCORES)))
    return np.concatenate(
        [res.results[c]["out"] for c in range(NCORES)], axis=0
    ).astype(np.float32)



# revision 2
# speedup vs baseline: 4.4142x; 4.4142x over previous
"""EngagementPredictor TRN2 kernel: 3-branch MHA + masked mean-pool + MLP.

Sharding: pure data-parallel - B=8 batch elements, one per NeuronCore;
weights replicated; no collectives. Each core computes its [2]-logit row.

Structure (per core):
  The pool weights are zero at masked-off positions AND masked keys get
  probability 0, so the whole computation collapses to attention over only
  the masked-in positions. Host compacts x to those S_c columns (padded to
  a multiple of 128, typically 640 of 1024); every projection/attention
  matmul shrinks accordingly. Padding keys project to K=V=0 (no bias added
  on device), so exp(score)=exp(0)=1 at pads; the softmax denominator
  (free from the Exp instruction's accum_out) is corrected by -n_pad, and
  padded V rows contribute exactly 0 to the pooled output.

  Scores are computed in [q-part, k-free] layout per (head, q-tile):
  PSUM <- QT_tile^T @ KT, exp on scalar engine with fused 1/sqrt(d) scale
  and per-partition denominator accum. The per-q context is never
  materialized: with c[q] = pw[q]/denom[q],
      pooled[d] = sum_k V[k,d] * gT[k],   gT[k] = sum_q c[q] exp[q,k]
  gT accumulates in PSUM via tiny N=1 matmuls with exp tiles stationary.
  The o-projection commutes with pooling and is folded into fus1 on the
  host (W_m = ow_m @ fus1_w[m-block]; ob folded into fus1_b), halving the
  matrix-vector tail.

  All matmul operands are bf16 (weights converted on host; Q/K/V/exp
  evacuated from PSUM as bf16) - full PE rate at any moving width and half
  the HBM traffic. PSUM accumulation stays fp32.
"""
import numpy as np
import ml_dtypes

import concourse.bass as bass
import concourse.tile as tile
from concourse import mybir
from concourse.bass_utils import run_bass_kernel_spmd

F32 = mybir.dt.float32
BF16 = mybir.dt.bfloat16
AF = mybir.ActivationFunctionType
ALU = mybir.AluOpType

P = 128
H = 1024
NT = H // P          # 8
NCORES = 8
MHAS = [("beh", 8), ("tmp", 4), ("pat", 4)]
LAG = 2              # scores-ahead-of-g software pipeline depth

_CACHE = {}


def _chunks(n, w=512):
    out = []
    o = 0
    while o < n:
        c = min(w, n - o)
        out.append((o, c))
        o += c
    return out


def _build_nc(S):
    T = S // P
    CH = _chunks(S)
    CH_H = _chunks(H)
    nc = bass.Bass()
    dram = {}

    def dp(name, shape, dt=BF16):
        dram[name] = nc.declare_dram_parameter(name, list(shape), dt,
                                               isOutput=False)

    dp("xT", (H, S))
    dp("pw", (P, T), F32)          # pool weight 1/cnt at real q, 0 at pads
    dp("nnp", (P, T), F32)         # -(S-cnt) at real q rows, 0 at pad rows
    for m, _ in MHAS:
        for wn in ("qw", "kw", "vw", "w1"):
            dp(f"{m}_{wn}", (H, H))
        dp(f"{m}_qb", (P, NT), F32)
        dp(f"{m}_vb", (P, NT), F32)
    dp("b1", (P, NT), F32)
    dp("fus2_w", (H, H // 2))
    dp("fus2_b", (P, 4), F32)
    dp("cls_w", (H // 2, 2))
    dp("cls_b", (1, 2), F32)
    out = nc.declare_dram_parameter("out", [1, 2], F32, isOutput=True)

    def r3(ap):  # [K, N] dram -> [P, K//P, N] partition-inner
        return ap[:].rearrange("(t p) n -> p t n", p=P)

    with tile.TileContext(nc) as tc, \
         nc.allow_low_precision(
             reason="bf16 storage/matmuls throughout; fp32 PSUM accumulation"):
        with tc.tile_pool(name="big", bufs=1) as big, \
             tc.tile_pool(name="qkv", bufs=2) as qkv, \
             tc.tile_pool(name="wstr", bufs=3) as wstr, \
             tc.tile_pool(name="expp", bufs=LAG + 2) as expp, \
             tc.tile_pool(name="small", bufs=1) as small, \
             tc.tile_pool(name="work", bufs=LAG + 2) as work, \
             tc.tile_pool(name="tails", bufs=1, space="PSUM") as ptails:

            xT = big.tile([P, NT, S], BF16, tag="xT")
            nc.sync.dma_start(xT[:], r3(dram["xT"]))
            pw = small.tile([P, T], F32, tag="pw")
            nc.sync.dma_start(pw[:], dram["pw"][:])
            nnp = small.tile([P, T], F32, tag="nnp")
            nc.sync.dma_start(nnp[:], dram["nnp"][:])

            h1acc = small.tile([P, NT], F32, tag="h1acc")
            nc.vector.memset(h1acc[:], 0.0)

            for mi, (m, nh) in enumerate(MHAS):
                d = H // nh
                ndt = d // P
                inv_sqrt_d = 1.0 / float(np.sqrt(d))

                qb = small.tile([P, NT], F32, tag=f"qb{mi}", name=f"qb{mi}")
                nc.sync.dma_start(qb[:], dram[f"{m}_qb"][:])
                vb = small.tile([P, NT], F32, tag=f"vb{mi}", name=f"vb{mi}")
                nc.sync.dma_start(vb[:], dram[f"{m}_vb"][:])

                QT = qkv.tile([P, NT, S], BF16, tag="QT", name=f"QT{mi}")
                KT = qkv.tile([P, NT, S], BF16, tag="KT", name=f"KT{mi}")
                V = qkv.tile([P, T, H], BF16, tag="V", name=f"V{mi}")

                # ---------- projections ----------
                with tc.tile_pool(name=f"pj{mi}", bufs=4, space="PSUM") as pj:
                    for wn, dst, bias in ((f"{m}_qw", QT, qb),
                                          (f"{m}_kw", KT, None)):
                        wt = wstr.tile([P, NT, H], BF16, tag="w",
                                       name=f"wt_{wn}")
                        nc.sync.dma_start(wt[:], r3(dram[wn]))
                        for ho in range(NT):
                            hsl = slice(ho * P, (ho + 1) * P)
                            for qo, qw_ in CH:
                                qsl = slice(qo, qo + qw_)
                                ps = pj.tile([P, 512], F32, tag="pj",
                                             name=f"pj_{wn}_{ho}_{qo}")
                                for ki in range(NT):
                                    nc.tensor.matmul(
                                        ps[:, :qw_],
                                        lhsT=wt[:, ki, hsl],
                                        rhs=xT[:, ki, qsl],
                                        start=(ki == 0), stop=(ki == NT - 1))
                                if bias is not None:
                                    nc.scalar.activation(
                                        dst[:, ho, qsl], ps[:, :qw_],
                                        AF.Identity, bias=bias[:, ho:ho + 1],
                                        scale=1.0)
                                else:
                                    nc.vector.tensor_copy(
                                        dst[:, ho, qsl], ps[:, :qw_])
                    wt = wstr.tile([P, NT, H], BF16, tag="w", name=f"wt_v{mi}")
                    nc.sync.dma_start(wt[:], r3(dram[f"{m}_vw"]))
                    for st in range(T):
                        ssl = slice(st * P, (st + 1) * P)
                        for ho2, hw2 in CH_H:
                            hsl = slice(ho2, ho2 + hw2)
                            ps = pj.tile([P, 512], F32, tag="pj",
                                         name=f"pjv{mi}_{st}_{ho2}")
                            for ki in range(NT):
                                nc.tensor.matmul(
                                    ps[:, :hw2],
                                    lhsT=xT[:, ki, ssl],
                                    rhs=wt[:, ki, hsl],
                                    start=(ki == 0), stop=(ki == NT - 1))
                            nc.vector.tensor_copy(V[:, st, hsl], ps[:, :hw2])

                # ---------- attention: scores/exp/denom -> c -> gT ----------
                gTall = small.tile([P, nh, T], BF16, tag=f"gT{mi}",
                                   name=f"gT{mi}")
                with tc.tile_pool(name=f"sc{mi}", bufs=2, space="PSUM") as psc, \
                     tc.tile_pool(name=f"g{mi}", bufs=2, space="PSUM") as pg:
                    gts = {}
                    pending = []

                    def emit_scores(h, qt):
                        qsl = slice(qt * P, (qt + 1) * P)
                        e2 = expp.tile([P, S], BF16, tag="e2",
                                       name=f"e2_{mi}_{h}_{qt}")
                        dp_ = work.tile([P, len(CH)], F32, tag="dp",
                                        name=f"dp_{mi}_{h}_{qt}")
                        for ci, (ko, kw_) in enumerate(CH):
                            ksl = slice(ko, ko + kw_)
                            ps = psc.tile([P, 512], F32, tag=f"sc{ci}",
                                          name=f"sc{mi}_{h}_{qt}_{ci}")
                            for dt in range(ndt):
                                hd = h * ndt + dt
                                nc.tensor.matmul(
                                    ps[:, :kw_],
                                    lhsT=QT[:, hd, qsl],
                                    rhs=KT[:, hd, ksl],
                                    start=(dt == 0), stop=(dt == ndt - 1))
                            nc.scalar.activation(
                                e2[:, ksl], ps[:, :kw_], AF.Exp,
                                scale=inv_sqrt_d,
                                accum_out=dp_[:, ci:ci + 1])
                        den = work.tile([P, 1], F32, tag="den",
                                        name=f"den_{mi}_{h}_{qt}")
                        if len(CH) > 1:
                            nc.vector.tensor_reduce(
                                den[:], dp_[:], axis=mybir.AxisListType.X,
                                op=ALU.add)
                            nc.vector.tensor_add(out=den[:], in0=den[:],
                                                 in1=nnp[:, qt:qt + 1])
                        else:
                            nc.vector.tensor_add(out=den[:], in0=dp_[:],
                                                 in1=nnp[:, qt:qt + 1])
                        rec = work.tile([P, 1], F32, tag="rec",
                                        name=f"rec_{mi}_{h}_{qt}")
                        nc.vector.reciprocal(rec[:], den[:])
                        cbf = work.tile([P, 1], BF16, tag="cbf",
                                        name=f"cbf_{mi}_{h}_{qt}")
                        nc.vector.tensor_mul(out=cbf[:], in0=rec[:],
                                             in1=pw[:, qt:qt + 1])
                        return e2, cbf

                    def emit_g(h, qt, e2, cbf):
                        gt = gts[h]
                        for kt in range(T):
                            nc.tensor.matmul(
                                gt[:, kt:kt + 1],
                                lhsT=e2[:, kt * P:(kt + 1) * P],
                                rhs=cbf[:],
                                start=(qt == 0), stop=(qt == T - 1))
                        if qt == T - 1:
                            nc.vector.tensor_copy(gTall[:, h, :], gt[:, :T])

                    for h in range(nh):
                        gts[h] = pg.tile([P, T], F32, tag="g",
                                         name=f"g{mi}_{h}")
                        for qt in range(T):
                            pending.append((h, qt) + emit_scores(h, qt))
                            if len(pending) > LAG:
                                emit_g(*pending.pop(0))
                    while pending:
                        emit_g(*pending.pop(0))

                    # ---------- pooled = V^T @ gT  (+vb) ----------
                    pooled_ps = ptails.tile([P, NT], F32, tag="tail",
                                            name=f"pooled{mi}")
                    for gdt in range(NT):
                        h = gdt // ndt
                        dsl = slice(gdt * P, (gdt + 1) * P)
                        for kt in range(T):
                            nc.tensor.matmul(
                                pooled_ps[:, gdt:gdt + 1],
                                lhsT=V[:, kt, dsl],
                                rhs=gTall[:, h, kt:kt + 1],
                                start=(kt == 0), stop=(kt == T - 1))
                    pooledm = small.tile([P, NT], BF16, tag=f"pm{mi}",
                                         name=f"pm{mi}")
                    nc.vector.tensor_add(out=pooledm[:], in0=pooled_ps[:],
                                         in1=vb[:])

                # ---------- h1 partial: h1acc += W_m^T @ pooledm ----------
                wt = wstr.tile([P, NT, H], BF16, tag="w", name=f"wt_w1_{mi}")
                nc.sync.dma_start(wt[:], r3(dram[f"{m}_w1"]))
                h1ps = ptails.tile([P, NT], F32, tag="tail", name=f"h1ps{mi}")
                for tg in range(NT):
                    tsl = slice(tg * P, (tg + 1) * P)
                    for ki in range(NT):
                        nc.tensor.matmul(
                            h1ps[:, tg:tg + 1],
                            lhsT=wt[:, ki, tsl],
                            rhs=pooledm[:, ki:ki + 1],
                            start=(ki == 0), stop=(ki == NT - 1))
                nc.vector.tensor_add(out=h1acc[:], in0=h1ps[:], in1=h1acc[:])

            # ---------- MLP tail ----------
            b1t = small.tile([P, NT], F32, tag="b1t")
            nc.sync.dma_start(b1t[:], dram["b1"][:])
            h1pre = small.tile([P, NT], F32, tag="h1pre")
            nc.vector.tensor_add(out=h1pre[:], in0=h1acc[:], in1=b1t[:])
            h1T = small.tile([P, NT], BF16, tag="h1T")
            nc.scalar.activation(h1T[:], h1pre[:], AF.Relu)

            w2 = wstr.tile([P, NT, H // 2], BF16, tag="w2", bufs=1)
            nc.sync.dma_start(w2[:], r3(dram["fus2_w"]))
            b2 = small.tile([P, 4], F32, tag="b2")
            nc.sync.dma_start(b2[:], dram["fus2_b"][:])
            ph2 = ptails.tile([P, 4], F32, tag="tail", name="ph2")
            for tg in range(4):
                tsl = slice(tg * P, (tg + 1) * P)
                for ki in range(NT):
                    nc.tensor.matmul(
                        ph2[:, tg:tg + 1],
                        lhsT=w2[:, ki, tsl],
                        rhs=h1T[:, ki:ki + 1],
                        start=(ki == 0), stop=(ki == NT - 1))
            h2pre = small.tile([P, 4], F32, tag="h2pre")
            nc.vector.tensor_add(out=h2pre[:], in0=ph2[:], in1=b2[:])
            h2T = small.tile([P, 4], BF16, tag="h2T")
            nc.scalar.activation(h2T[:], h2pre[:], AF.Relu)

            cw = small.tile([P, 4, 2], BF16, tag="cw")
            nc.sync.dma_start(cw[:], r3(dram["cls_w"]))
            plg = ptails.tile([1, 2], F32, tag="tail", name="plg")
            for ki in range(4):
                nc.tensor.matmul(plg[:],
                                 lhsT=h2T[:, ki:ki + 1],
                                 rhs=cw[:, ki],
                                 start=(ki == 0), stop=(ki == 3))
            cb = small.tile([1, 2], F32, tag="cb")
            nc.sync.dma_start(cb[:], dram["cls_b"][:])
            lg = small.tile([1, 2], F32, tag="lgsb")
            nc.vector.tensor_add(out=lg[:], in0=plg[:], in1=cb[:])
            nc.sync.dma_start(out[:], lg[:])

    _split_multi_waits(nc)
    return nc


def _split_multi_waits(nc, max_on_inst=1, max_on_evsem=2):
    """This walrus build caps sync waits per instruction at 1 (2 for
    EventSemaphore); Tile attaches one wait per dependent proc. Spill excess
    waits onto pure-wait EventSemaphores inserted before, on the same engine -
    the engine blocks on each condition in sequence, so semantics match."""
    for f in nc.m.functions:
        for bb in f.blocks:
            insts = list(bb.instructions)
            new = []
            changed = False
            for ins in insts:
                si = ins.sync_info
                if si is not None:
                    waits = list(si.on_wait)
                    cap = (max_on_evsem
                           if isinstance(ins, mybir.InstEventSemaphore)
                           else max_on_inst)
                    if len(waits) > cap:
                        spill = waits[:-cap]
                        keep = waits[-cap:]
                        k = 0
                        while spill:
                            chunk = spill[:max_on_evsem]
                            spill = spill[max_on_evsem:]
                            new.append(mybir.InstEventSemaphore(
                                name=f"{ins.name}-wspill{k}",
                                engine=ins.engine, ins=[], outs=[],
                                sync_info=mybir.SyncInfo(on_wait=chunk,
                                                         on_update=[])))
                            k += 1
                        ins.sync_info = mybir.SyncInfo(
                            on_wait=keep, on_update=list(si.on_update))
                        changed = True
                new.append(ins)
            if changed:
                bb.instructions = new


def _get_nc(S):
    if S not in _CACHE:
        _CACHE[S] = _build_nc(S)
    return _CACHE[S]


def _prep(inputs):
    f32 = np.float32
    bf16 = ml_dtypes.bfloat16

    def cm(b, nt=NT):  # [nt*P] bias -> [P, nt] partition-inner
        return np.ascontiguousarray(
            np.asarray(b, f32).reshape(nt, P).T)

    mask = np.asarray(inputs["attention_mask"])
    cnts = mask.sum(axis=1)
    S = int(max(P, -(-int(cnts.max()) // P) * P))
    T = S // P

    w1full = np.asarray(inputs["fus1_w"], f32)
    b1 = np.asarray(inputs["fus1_b"], f32).copy()
    shared = {}
    for mi, (m, _) in enumerate(MHAS):
        for wn in ("qw", "kw", "vw"):
            shared[f"{m}_{wn}"] = np.asarray(
                inputs[f"{m}_{wn}"], f32).astype(bf16)
        w1b = w1full[mi * H:(mi + 1) * H]
        ow = np.asarray(inputs[f"{m}_ow"], f32)
        shared[f"{m}_w1"] = (ow @ w1b).astype(bf16)
        b1 += np.asarray(inputs[f"{m}_ob"], f32) @ w1b
        shared[f"{m}_qb"] = cm(inputs[f"{m}_qb"])
        shared[f"{m}_vb"] = cm(inputs[f"{m}_vb"])
    shared["b1"] = cm(b1)
    shared["fus2_w"] = np.asarray(inputs["fus2_w"], f32).astype(bf16)
    shared["fus2_b"] = cm(inputs["fus2_b"], 4)
    shared["cls_w"] = np.asarray(inputs["cls_w"], f32).astype(bf16)
    shared["cls_b"] = np.asarray(inputs["cls_b"], f32).reshape(1, 2)

    x = np.asarray(inputs["hidden_states"], f32)
    in_maps = []
    for c in range(NCORES):
        im = dict(shared)
        sel = np.flatnonzero(mask[c])
        cnt = len(sel)
        xc = np.zeros((S, H), f32)
        xc[:cnt] = x[c][sel]
        im["xT"] = np.ascontiguousarray(xc.T).astype(bf16)
        pwv = np.zeros(S, f32)
        pwv[:cnt] = 1.0 / max(cnt, 1)
        im["pw"] = np.ascontiguousarray(pwv.reshape(T, P).T)
        # -n_pad denominator fix on real q rows; 0 on pad rows so the
        # (unused) reciprocal there can't hit a zero denominator
        nnpv = np.zeros(S, f32)
        nnpv[:cnt] = -float(S - cnt)
        im["nnp"] = np.ascontiguousarray(nnpv.reshape(T, P).T)
        in_maps.append(im)
    return S, in_maps


def kernel(**inputs) -> np.ndarray:
    S, in_maps = _prep(inputs)
    nc = _get_nc(S)
    res = run_bass_kernel_spmd(nc, in_maps, core_ids=list(range(NCORES)))
    return np.concatenate(
        [res.results[c]["out"] for c in range(NCORES)], axis=0
    ).astype(np.float32)
